# revision 57
# baseline (speedup 1.0000x reference)
import os
import sys
import contextlib
import numpy as np

sys.path.insert(0, "/opt/trn_rl_repo")

import concourse.bass as bass  # noqa: E402
import concourse.tile as tile  # noqa: E402
from concourse import bacc, mybir  # noqa: E402
from concourse.bass_utils import run_bass_kernel_spmd  # noqa: E402
from concourse.masks import make_identity  # noqa: E402

FP = mybir.dt.float32
BF = mybir.dt.bfloat16
FR = mybir.dt.float32r
AF = mybir.ActivationFunctionType
ALU = mybir.AluOpType
AX = mybir.AxisListType

# Problem dims (hardcoded; kernel.py must be self-contained)
B, S_FULL, I, H, N, W = 32, 128, 512, 1024, 16384, 128
TH = 3 * H          # 3072
NCORES = 8
BC = B // NCORES    # 4 batches per core
KH = H // 128       # 8
KI = I // 128       # 4
NT = TH // 512      # 6
NC128 = N // 128    # 128 chunks of memory rows
EPS = 1e-8


def _transpose_from_dram(nc, tc, pool_tmp, src_view, dst_tile, n_mchunks,
                         n_kchunks, ident, src_cols):
    """dst[p, k, m] = src[m, k*128+p]; src is [n_mchunks*128, src_cols] in
    DRAM, dst an SBUF tile [128, n_kchunks, n_mchunks*128] (any dtype; the
    PSUM->SBUF copy casts). Uses its own psum pool (2 banks), freed on
    return."""
    with tc.tile_pool(name="tr_ps", bufs=2, space="PSUM") as pps:
        for mj in range(n_mchunks):
            nat = pool_tmp.tile([128, src_cols], FP, tag="tr_nat")
            nc.sync.dma_start(out=nat[:], in_=src_view[mj * 128:(mj + 1) * 128, :])
            for ki in range(n_kchunks):
                tp = pps.tile([128, 128], FP, tag="tr_ps")
                nc.tensor.transpose(tp[:], nat[:, ki * 128:(ki + 1) * 128], ident[:])
                nc.vector.tensor_copy(
                    out=dst_tile[:, ki, mj * 128:(mj + 1) * 128], in_=tp[:])


def _emit_gru_step(nc, st, t):
    """One GRU step for layer state dict `st`.

    Fat layouts (partition = 32*g + b, only b<4 rows valid):
      - r,z gates: [128, 512], group g covers gate cols [512g, 512g+512)
        (g 0,1 = r over H; g 2,3 = z over H).
      - n gate / h state: [128, 256], group g covers H cols [256g, 256g+256).
    Col-tiled matmuls (tile_position=(0, 32g)) run the 4 groups concurrently
    on HW. gi arrives pre-scattered in the same fat layouts.
    """
    Tg = st["Tg"]
    if t % Tg == 0:
        gir = st["pgi"].tile([128, Tg, 256], BF, tag="gir")
        giz = st["pgi"].tile([128, Tg, 256], BF, tag="giz")
        gin = st["pgi"].tile([128, Tg, 256], BF, tag="gin")
        vR, vZ, vB = st["gi_view"](t)
        nc.sync.dma_start(out=gir[:], in_=vR)
        nc.sync.dma_start(out=giz[:], in_=vZ)
        nc.sync.dma_start(out=gin[:], in_=vB)
        st["gi_cur"] = (gir, giz, gin)
    giR = st["gi_cur"][0][:, t % Tg, :]
    giZ = st["gi_cur"][1][:, t % Tg, :]
    giN = st["gi_cur"][2][:, t % Tg, :]
    hT, h, whhT = st["hT"], st["h"], st["whhT"]
    pew = st["pew"]

    pgR = st["psR"].tile([128, 256], FP, tag="pgR")
    pgZ = st["psZ"].tile([128, 256], FP, tag="pgZ")
    pgN = st["psN"].tile([128, 256], FP, tag="pgN")
    # skip_group_check: the 4 col-groups run concurrent accumulation groups in
    # one psum bank at different base partitions; CoreSim's zero-region
    # tracker ignores base partition and would falsely flag a conflict (HW
    # has_written bits are per (partition, element) — verified on device).
    # hT is padded to 32 cols (zeros beyond BC) so every psum partition gets
    # written — downstream full-tile ops then read only defined data.
    for gate, pg in ((0, pgR), (1, pgZ), (2, pgN)):
        laststop = (st["bhh_n"] is None) or gate != 2
        for k in range(KH):
            for g in range(4):
                c0 = 1024 * gate + 256 * g
                nc.tensor.matmul(pg[32 * g:32 * g + 32, :], hT[:, k, :],
                                 whhT[:, k, c0:c0 + 256],
                                 start=(k == 0),
                                 stop=(k == KH - 1 and laststop),
                                 tile_position=(0, 32 * g),
                                 skip_group_check=True)
    if st["bhh_n"] is not None:
        for g in range(4):
            nc.tensor.matmul(pgN[32 * g:32 * g + 32, :], st["ones1x128"][:, 0:32],
                             st["bhh_n"][:, 256 * g:256 * (g + 1)],
                             start=False, stop=True, tile_position=(0, 32 * g),
                             skip_group_check=True)

    if st.get("substep", 9) < 2:
        return
    rp = pew.tile([128, 256], BF, tag="rp")
    nc.vector.tensor_add(out=rp[:], in0=pgR[:], in1=giR)
    r_f = pew.tile([128, 256], BF, tag="r_f")
    nc.scalar.activation(out=r_f[:], in_=rp[:], func=AF.Sigmoid)
    zp = pew.tile([128, 256], BF, tag="zp")
    nc.vector.tensor_add(out=zp[:], in0=pgZ[:], in1=giZ)
    z_f = pew.tile([128, 256], BF, tag="z_f")
    nc.scalar.activation(out=z_f[:], in_=zp[:], func=AF.Sigmoid)

    npf = pew.tile([128, 256], BF, tag="npf")
    nc.vector.tensor_mul(out=npf[:], in0=pgN[:], in1=r_f[:])
    nf = pew.tile([128, 256], BF, tag="nf")
    nc.vector.tensor_add(out=nf[:], in0=npf[:], in1=giN)
    n_f = pew.tile([128, 256], BF, tag="n_f")
    nc.scalar.activation(out=n_f[:], in_=nf[:], func=AF.Tanh)

    hmn = pew.tile([128, 256], BF, tag="hmn")
    nc.vector.tensor_tensor(out=hmn[:], in0=h[:], in1=n_f[:], op=ALU.subtract)
    h2 = st["phh"].tile([128, 256], BF, tag="h")
    nc.vector.tensor_mul(out=h2[:], in0=hmn[:], in1=z_f[:])
    nc.vector.tensor_add(out=h2[:], in0=h2[:], in1=n_f[:])

    st["h"] = h2
    st["h2_pend"] = h2


def _emit_gru_step_tail(nc, st, t):
    """Transpose h2 for the next step's stationary. Emitted after the other
    layer's matmuls so the PE isn't stalled waiting on this layer's chain."""
    if st.get("substep", 9) < 3 or "h2_pend" not in st:
        return
    h2 = st.pop("h2_pend")
    pew = st["pew"]
    # fat -> natural copy; row-positioned transposes straight off the fat
    # layout read garbage on HW when mixed with col-tiled matmuls, so keep
    # all transposes at base partition 0
    h2n = pew.tile([BC, 1024], BF, tag="h2n")
    for q in range(4):
        nc.vector.tensor_copy(out=h2n[:, 256 * q:256 * (q + 1)],
                              in_=h2[32 * q:32 * q + 4, :])
    htp = st["pht"].tile([128, KH, BC], BF, tag="htp")
    for j in range(KH):
        nc.tensor.transpose(htp[:, j, :], h2n[:, 128 * j:128 * (j + 1)],
                            st["identbf"][0:BC, 0:BC])
    hT2 = st["hT_slots"][(t + 1) % 2]
    nc.vector.tensor_copy(out=hT2[:, :, 0:BC], in_=htp[:])
    if st.get("y0blk") is not None and st.get("substep", 9) >= 5:
        nc.vector.tensor_copy(out=st["y0blk"][:, :, :, t % st["TBLK"]],
                              in_=htp[:])
    if st.get("substep", 9) >= 4:
        st["hT"] = hT2


def build_nc(S=S_FULL, nonzero_biases=(), debug=False, stop_phase=7):
    nzb = set(nonzero_biases)
    nc = bacc.Bacc("TRN2", target_bir_lowering=False, debug=False,
                   num_devices=NCORES)
    Tg = min(4, S)
    TBLK = min(16, S)
    assert S % TBLK == 0 and TBLK % Tg == 0

    # ---- DRAM I/O ----
    x_d = nc.declare_dram_parameter("x", [BC, S_FULL, I], FP, isOutput=False)
    mem_d = nc.declare_dram_parameter("memory", [BC, N, W], FP, isOutput=False)
    wprev_d = nc.declare_dram_parameter("w_prev", [BC, N], FP, isOutput=False)
    Wih0_d = nc.declare_dram_parameter("W_ih0", [TH, I], FP, isOutput=False)
    Whh0_d = nc.declare_dram_parameter("W_hh0", [TH, H], FP, isOutput=False)
    Wih1_d = nc.declare_dram_parameter("W_ih1", [TH, H], FP, isOutput=False)
    Whh1_d = nc.declare_dram_parameter("W_hh1", [TH, H], FP, isOutput=False)
    Wk_d = nc.declare_dram_parameter("Wk", [H, W], FP, isOutput=False)
    Wbeta_d = nc.declare_dram_parameter("Wbeta", [H, 1], FP, isOutput=False)
    Wg_d = nc.declare_dram_parameter("Wg", [H, 1], FP, isOutput=False)
    Wgamma_d = nc.declare_dram_parameter("Wgamma", [H, 1], FP, isOutput=False)
    We_d = nc.declare_dram_parameter("We", [H, W], FP, isOutput=False)
    Wa_d = nc.declare_dram_parameter("Wa", [H, W], FP, isOutput=False)
    Wout_d = nc.declare_dram_parameter("Wout", [I, H + W], FP, isOutput=False)
    bias_d = {}
    for nm, sz in [("bih0", TH), ("bhh0", TH), ("bih1", TH), ("bhh1", TH),
                   ("bk", W), ("bbeta", 1), ("bg", 1), ("bgamma", 1),
                   ("be", W), ("ba", W), ("bout", I)]:
        if nm in nzb:
            bias_d[nm] = nc.declare_dram_parameter(nm, [1, sz], FP,
                                                   isOutput=False)
    out_d = nc.declare_dram_parameter("out", [BC, I], FP, isOutput=True)
    if debug:
        dbg_h1_d = nc.declare_dram_parameter("dbg_h1", [BC, H], FP, isOutput=True)
        dbg_head_d = nc.declare_dram_parameter("dbg_head", [BC, 387], FP,
                                               isOutput=True)
        dbg_w_d = nc.declare_dram_parameter("dbg_w", [128, NC128], FP,
                                            isOutput=True)
        dbg_r_d = nc.declare_dram_parameter("dbg_r", [128, BC], FP, isOutput=True)

    # gi scratch in "fat" layout: [t, g, slot32, 256] per gate (r/z/n) so a
    # step's gate rows land at partitions 32*g+b directly (slots 4..31 unused).
    gi0_gates = [nc.dram_tensor(f"gi0{gn}_scr", [S_FULL, 4, 32, 256], BF)
                 for gn in "rzn"]
    gi1_gates = [[nc.dram_tensor(f"gi1{gn}_scr{i}", [TBLK, 4, 32, 256], BF)
                  for gn in "rzn"] for i in range(S // TBLK)]
    membf_d = nc.dram_tensor("membf_scr", [BC, N, W], BF)
    xbf_d = nc.dram_tensor("xbf_scr", [BC * S_FULL, I], BF)
    wih0bf_d = nc.dram_tensor("wih0bf_scr", [TH, I], BF)
    woutbf_d = nc.dram_tensor("woutbf_scr", [I, H + W], BF)
    whh0bf_d = nc.dram_tensor("whh0bf_scr", [TH, H], BF)
    whh1bf_d = nc.dram_tensor("whh1bf_scr", [TH, H], BF)
    wih1bf_d = nc.dram_tensor("wih1bf_scr", [TH, H], BF)

    with tile.TileContext(nc) as tc, contextlib.ExitStack() as top:
        const = top.enter_context(tc.tile_pool(name="const", bufs=1))
        ptmp = top.enter_context(tc.tile_pool(name="ptmp", bufs=2))

        ident = const.tile([128, 128], FP)
        make_identity(nc, ident[:])
        ones1x128 = const.tile([1, 128], FP)
        nc.vector.memset(ones1x128[:], 1.0)
        ones128 = const.tile([128, 1], FP)
        nc.vector.memset(ones128[:], 1.0)
        onesbf = const.tile([128, 1], BF)
        nc.vector.memset(onesbf[:], 1.0)
        eps128 = const.tile([128, 1], FP)
        nc.vector.memset(eps128[:], EPS)
        ones1bc = const.tile([1, BC], FP)
        nc.vector.memset(ones1bc[:], 1.0)
        identbf = const.tile([128, 128], BF)
        nc.vector.tensor_copy(out=identbf[:], in_=ident[:])

        # early DRAM->DRAM casts (gpsimd queue; overlap with A0/GRU)
        nc.gpsimd.dma_start(out=xbf_d[:],
                            in_=x_d[:].rearrange("b s i -> (b s) i"))
        nc.gpsimd.dma_start(out=wih0bf_d[:], in_=Wih0_d[:])
        nc.gpsimd.dma_start(out=whh0bf_d[:], in_=Whh0_d[:])
        nc.gpsimd.dma_start(out=whh1bf_d[:], in_=Whh1_d[:])
        nc.gpsimd.dma_start(out=wih1bf_d[:], in_=Wih1_d[:])
        nc.gpsimd.dma_start(out=woutbf_d[:], in_=Wout_d[:])
        for b in range(BC):
            nc.gpsimd.dma_start(out=membf_d[b], in_=mem_d[b])

        bias_t = {}
        for nm in bias_d:
            t = const.tile([1, bias_d[nm].shape[1]], FP, tag=f"b_{nm}")
            nc.sync.dma_start(out=t[:], in_=bias_d[nm][:])
            bias_t[nm] = t

        def bias_mm(psum_ap, src_ap, nrows):
            nc.tensor.matmul(psum_ap, ones1x128[:, 0:nrows], src_ap,
                             start=False, stop=True)

        # ---------------- phase A0: gi0 = x @ W_ih0.T (+ biases) -------------
        with contextlib.ExitStack() as ph:
            pw = ph.enter_context(tc.tile_pool(name="pw_a0", bufs=1))
            ptb = ph.enter_context(tc.tile_pool(name="ptmp_a0", bufs=2))
            xT = pw.tile([128, KI, BC * S_FULL], BF)
            wT = pw.tile([128, KI, TH], BF)
            for ki in range(KI):
                nc.scalar.dma_start_transpose(
                    xT[:, ki, :], xbf_d[:, ki * 128:(ki + 1) * 128])
                nc.scalar.dma_start_transpose(
                    wT[:, ki, :], wih0bf_d[:, ki * 128:(ki + 1) * 128])

            bsum = None
            if "bih0" in nzb or "bhh0" in nzb:
                bsum = pw.tile([1, TH], FP, tag="bsum0")
                nc.vector.memset(bsum[:], 0.0)
                if "bih0" in nzb:
                    nc.vector.tensor_copy(out=bsum[:], in_=bias_t["bih0"][:])
                if "bhh0" in nzb:
                    nc.vector.tensor_add(out=bsum[:, 0:2048],
                                         in0=bsum[:, 0:2048],
                                         in1=bias_t["bhh0"][:, 0:2048])

            with tc.tile_pool(name="pps_a0", bufs=1, space="PSUM") as pps:
                for rj in range(BC * S_FULL // 128):
                    pg = pps.tile([128, NT, 512], FP, tag="pg_a0")
                    for nt in range(NT):
                        for ki in range(KI):
                            nc.tensor.matmul(
                                pg[:, nt, :], xT[:, ki, rj * 128:(rj + 1) * 128],
                                wT[:, ki, nt * 512:(nt + 1) * 512],
                                start=(ki == 0),
                                stop=(ki == KI - 1 and bsum is None))
                        if bsum is not None:
                            bias_mm(pg[:, nt, :],
                                    bsum[:, nt * 512:(nt + 1) * 512], 128)
                    gs = ptb.tile([128, TH], BF, tag="gs_a0")
                    nc.vector.tensor_copy(out=gs[:],
                                          in_=pg[:].rearrange("p n x -> p (n x)"))
                    # rj == b here (BC*S_FULL//128 == BC): scatter into fat gi0
                    for gate in range(3):
                        for g in range(4):
                            c0 = 1024 * gate + 256 * g
                            nc.sync.dma_start(out=gi0_gates[gate][:, g, rj, :],
                                              in_=gs[:, c0:c0 + 256])

        # ------------- interleaved recurrences L0 + L1 -----------------------
        pkeep = top.enter_context(tc.tile_pool(name="pkeep", bufs=1))
        h1T_keep = pkeep.tile([128, KH, BC], BF)
        h1_keep = pkeep.tile([BC, H], FP)
        if stop_phase >= 2:
          with contextlib.ExitStack() as ph:
            pw = ph.enter_context(tc.tile_pool(name="pw_gru", bufs=1))
            whh0T = pw.tile([128, KH, TH], BF, tag="whh0T")
            whh1T = pw.tile([128, KH, TH], BF, tag="whh1T")
            wih1T = pw.tile([128, KH, TH], BF, tag="wih1T")
            for k in range(KH):
                nc.scalar.dma_start_transpose(
                    whh0T[:, k, :], whh0bf_d[:, k * 128:(k + 1) * 128])
                nc.scalar.dma_start_transpose(
                    whh1T[:, k, :], whh1bf_d[:, k * 128:(k + 1) * 128])
                nc.scalar.dma_start_transpose(
                    wih1T[:, k, :], wih1bf_d[:, k * 128:(k + 1) * 128])

            bsum1 = None
            if "bih1" in nzb or "bhh1" in nzb:
                bsum1 = pw.tile([1, TH], FP, tag="bsum1")
                nc.vector.memset(bsum1[:], 0.0)
                if "bih1" in nzb:
                    nc.vector.tensor_copy(out=bsum1[:], in_=bias_t["bih1"][:])
                if "bhh1" in nzb:
                    nc.vector.tensor_add(out=bsum1[:, 0:2048],
                                         in0=bsum1[:, 0:2048],
                                         in1=bias_t["bhh1"][:, 0:2048])
            bhh0n = bhh1n = None
            if "bhh0" in nzb:
                bhh0n = pw.tile([1, 1024], FP, tag="bhh0n")
                nc.vector.tensor_copy(out=bhh0n[:], in_=bias_t["bhh0"][:, 2048:TH])
            if "bhh1" in nzb:
                bhh1n = pw.tile([1, 1024], FP, tag="bhh1n")
                nc.vector.tensor_copy(out=bhh1n[:], in_=bias_t["bhh1"][:, 2048:TH])

            py0 = ph.enter_context(tc.tile_pool(name="py0", bufs=2))
            psG = ph.enter_context(tc.tile_pool(name="psG", bufs=1, space="PSUM"))
            pgo = ph.enter_context(tc.tile_pool(name="pgo", bufs=2))

            def gi0_view(t):
                return tuple(d[t:t + Tg].rearrange("t g q x -> (g q) t x")
                             for d in gi0_gates)

            def gi1_view(t):
                blk, lo = t // TBLK, t % TBLK
                return tuple(d[lo:lo + Tg].rearrange("t g q x -> (g q) t x")
                             for d in gi1_gates[blk])

            pht_shared = ph.enter_context(
                tc.tile_pool(name="pht", bufs=1, space="PSUM"))
            sts = []
            for li, (whhT, gi_view, bhh_n) in enumerate(
                    [(whh0T, gi0_view, bhh0n), (whh1T, gi1_view, bhh1n)]):
                st = {
                    "Tg": Tg, "TBLK": TBLK, "gi_view": gi_view, "whhT": whhT,
                    "bhh_n": bhh_n, "ident": ident, "identbf": identbf,
                    "ones1bc": ones1bc,
                    "psR": ph.enter_context(
                        tc.tile_pool(name=f"psR{li}", bufs=1, space="PSUM")),
                    "psZ": ph.enter_context(
                        tc.tile_pool(name=f"psZ{li}", bufs=1, space="PSUM")),
                    "psN": ph.enter_context(
                        tc.tile_pool(name=f"psN{li}", bufs=1, space="PSUM")),
                    "pht": pht_shared,
                    "pew": ph.enter_context(tc.tile_pool(name=f"pew{li}", bufs=1)),
                    "pgi": ph.enter_context(tc.tile_pool(name=f"pgi{li}", bufs=2)),
                    "phh": ph.enter_context(tc.tile_pool(name=f"phh{li}", bufs=2)),
                }
                st["ones1x128"] = ones1x128
                st["substep"] = int(os.environ.get("BASSGRU_SUBSTEP", "9"))
                h = st["phh"].tile([128, 256], BF, tag="h")
                nc.vector.memset(h[:], 0.0)
                # persistent double-buffered transposed-h slots, padded to 32
                # cols; the zero padding makes the matmuls write every psum
                # partition with defined values
                slots = []
                for sl in range(2):
                    hTs = pw.tile([128, KH, 32], BF, tag=f"hTs{li}_{sl}")
                    nc.vector.memset(hTs[:], 0.0)
                    slots.append(hTs)
                st["hT_slots"] = slots
                st["h"], st["hT"] = h, slots[0]
                sts.append(st)

            def emit_gi1_block(blk, y0blk):
                BT = BC * TBLK
                for nt in range(NT):
                    pg = psG.tile([BT, 512], FP, tag="pgG")
                    for k in range(KH):
                        nc.tensor.matmul(pg[:], y0blk[:, k, :, :],
                                         wih1T[:, k, 512 * nt:512 * (nt + 1)],
                                         start=(k == 0),
                                         stop=(k == KH - 1 and bsum1 is None))
                    if bsum1 is not None:
                        bias_mm(pg[:], bsum1[:, nt * 512:(nt + 1) * 512], BT)
                    gs = pgo.tile([BT, 512], BF, tag="gsG")
                    nc.vector.tensor_copy(out=gs[:], in_=pg[:])
                    gate, gbase = nt // 2, 2 * (nt % 2)
                    for half in range(2):
                        nc.sync.dma_start(
                            out=gi1_gates[blk][gate][:, gbase + half, 0:BC, :]
                            .rearrange("t b x -> b t x"),
                            in_=gs[:, 256 * half:256 * (half + 1)])

            sub = int(os.environ.get("BASSGRU_SUB", "9"))
            y0blk = None
            for t in range(S):
                if t % TBLK == 0:
                    y0blk = py0.tile([128, KH, BC, TBLK], BF, tag="y0blk")
                    sts[0]["y0blk"] = y0blk
                if sub >= 2:
                    _emit_gru_step(nc, sts[0], t)
                if stop_phase >= 3 and t >= TBLK:
                    _emit_gru_step(nc, sts[1], t - TBLK)
                if sub >= 2:
                    _emit_gru_step_tail(nc, sts[0], t)
                if stop_phase >= 3 and t >= TBLK:
                    _emit_gru_step_tail(nc, sts[1], t - TBLK)
                if (t + 1) % TBLK == 0 and sub >= 3:
                    emit_gi1_block(t // TBLK, y0blk)
            if stop_phase >= 3:
                for t in range(S - TBLK, S):
                    _emit_gru_step(nc, sts[1], t)
                    _emit_gru_step_tail(nc, sts[1], t)
                nc.vector.tensor_copy(out=h1T_keep[:],
                                      in_=sts[1]["hT"][:, :, 0:BC])
                for q in range(4):
                    nc.vector.tensor_copy(
                        out=h1_keep[:, 256 * q:256 * (q + 1)],
                        in_=sts[1]["h"][32 * q:32 * q + 4, :])
                if debug:
                    nc.sync.dma_start(out=dbg_h1_d[:], in_=h1_keep[:])

        # ---------------- phase H: NTM head ---------------------------------
        if stop_phase >= 5:
          hp = top.enter_context(tc.tile_pool(name="hp", bufs=1))
          ph_psum_stack = contextlib.ExitStack()
          pps_h = ph_psum_stack.enter_context(
              tc.tile_pool(name="pps_h", bufs=2, space="PSUM"))

          wcatf = hp.tile([128, KH, 512], FP, tag="wcatf")
          nc.vector.memset(wcatf[:], 0.0)
          nc.sync.dma_start(out=wcatf[:, :, 0:128],
                            in_=Wk_d[:].rearrange("(k p) w -> p k w", p=128))
          nc.sync.dma_start(out=wcatf[:, :, 128:256],
                            in_=We_d[:].rearrange("(k p) w -> p k w", p=128))
          nc.sync.dma_start(out=wcatf[:, :, 256:384],
                            in_=Wa_d[:].rearrange("(k p) w -> p k w", p=128))
          nc.sync.dma_start(out=wcatf[:, :, 384:385],
                            in_=Wbeta_d[:].rearrange("(k p) w -> p k w", p=128))
          nc.sync.dma_start(out=wcatf[:, :, 385:386],
                            in_=Wg_d[:].rearrange("(k p) w -> p k w", p=128))
          nc.sync.dma_start(out=wcatf[:, :, 386:387],
                            in_=Wgamma_d[:].rearrange("(k p) w -> p k w", p=128))
          wcat = hp.tile([128, KH, 512], BF, tag="wcat")
          nc.vector.tensor_copy(out=wcat[:], in_=wcatf[:])

          bcat = None
          if any(nm in nzb for nm in ("bk", "bbeta", "bg", "bgamma", "be", "ba")):
              bcat = hp.tile([1, 512], FP, tag="bcat")
              nc.vector.memset(bcat[:], 0.0)
              for nm, lo, hi in [("bk", 0, 128), ("be", 128, 256), ("ba", 256, 384),
                                 ("bbeta", 384, 385), ("bg", 385, 386),
                                 ("bgamma", 386, 387)]:
                  if nm in nzb:
                      nc.vector.tensor_copy(out=bcat[:, lo:hi], in_=bias_t[nm][:])

          phead = pps_h.tile([BC, 512], FP, tag="hps")
          for k in range(KH):
              nc.tensor.matmul(phead[:], h1T_keep[:, k, :], wcat[:, k, :],
                               start=(k == 0),
                               stop=(k == KH - 1 and bcat is None))
          if bcat is not None:
              bias_mm(phead[:], bcat[:], BC)
          head = hp.tile([BC, 512], FP, tag="head")
          nc.vector.tensor_copy(out=head[:], in_=phead[:])
          if debug:
              nc.sync.dma_start(out=dbg_head_d[:], in_=head[:, 0:387])

          e_t = hp.tile([BC, 128], FP, tag="e_t")
          nc.scalar.activation(out=e_t[:], in_=head[:, 128:256], func=AF.Sigmoid)
          a_t = hp.tile([BC, 128], FP, tag="a_t")
          nc.scalar.activation(out=a_t[:], in_=head[:, 256:384], func=AF.Tanh)
          # softplus(x) = ln(1 + exp(x)) for beta and gamma (no Softplus table)
          bg2 = hp.tile([BC, 2], FP, tag="bg2")
          nc.scalar.activation(out=bg2[:, 0:1], in_=head[:, 384:385], func=AF.Exp)
          nc.scalar.activation(out=bg2[:, 1:2], in_=head[:, 386:387], func=AF.Exp)
          nc.vector.tensor_scalar_add(bg2[:], bg2[:], 1.0)
          bg2l = hp.tile([BC, 2], FP, tag="bg2l")
          nc.scalar.activation(out=bg2l[:], in_=bg2[:], func=AF.Ln)
          beta_t = hp.tile([BC, 1], FP, tag="beta_t")
          nc.vector.tensor_copy(out=beta_t[:], in_=bg2l[:, 0:1])
          g_t = hp.tile([BC, 1], FP, tag="g_t")
          nc.scalar.activation(out=g_t[:], in_=head[:, 385:386], func=AF.Sigmoid)
          gam_t = hp.tile([BC, 1], FP, tag="gam_t")
          nc.vector.tensor_scalar_add(gam_t[:], bg2l[:, 1:2], 1.0)

          k_t = hp.tile([BC, 128], FP, tag="k_t")
          nc.vector.tensor_copy(out=k_t[:], in_=head[:, 0:128])
          kn2 = hp.tile([BC, 1], FP, tag="kn2")
          ksc = hp.tile([BC, 128], FP, tag="ksc")
          nc.vector.tensor_mul(out=ksc[:], in0=k_t[:], in1=k_t[:])
          nc.vector.tensor_reduce(out=kn2[:], in_=ksc[:], axis=AX.X,
                                  op=ALU.add)
          knrm = hp.tile([BC, 1], FP, tag="knrm")
          nc.scalar.activation(out=knrm[:], in_=kn2[:], func=AF.Sqrt)
          nc.vector.tensor_scalar_add(knrm[:], knrm[:], EPS)
          krec = hp.tile([BC, 1], FP, tag="krec")
          nc.vector.reciprocal(out=krec[:], in_=knrm[:])
          nc.vector.tensor_scalar_mul(krec[:], krec[:], beta_t[:])
          kb = hp.tile([BC, 128], FP, tag="kb")
          nc.vector.tensor_scalar_mul(kb[:], k_t[:], krec[:])

          def tr_small(src_ap, nrows, ncols, tag):
              tp = pps_h.tile([ncols, nrows], FP, tag="hps_tr")
              nc.tensor.transpose(tp[:], src_ap, ident[0:nrows, 0:nrows])
              dst = hp.tile([ncols, nrows], FP, tag=tag)
              nc.vector.tensor_copy(out=dst[:], in_=tp[:])
              return dst

          kbT = tr_small(kb[:], BC, 128, "kbT")
          eT = tr_small(e_t[:], BC, 128, "eT")
          aT = tr_small(a_t[:], BC, 128, "aT")
          gT = tr_small(g_t[:], BC, 1, "gT")
          gamT = tr_small(gam_t[:], BC, 1, "gamT")

          khl = hp.tile([128, 2 * BC], BF, tag="khl")
          nc.vector.tensor_copy(out=khl[:, 0:BC], in_=kbT[:])
          klo = hp.tile([128, BC], FP, tag="klo")
          nc.vector.tensor_tensor(out=klo[:], in0=kbT[:], in1=khl[:, 0:BC],
                                  op=ALU.subtract)
          nc.vector.tensor_copy(out=khl[:, BC:2 * BC], in_=klo[:])

          combT = pkeep.tile([128, KH + 1, BC], BF, tag="combT")
          nc.vector.tensor_copy(out=combT[:, 0:KH, :], in_=h1T_keep[:])

          ph_psum_stack.close()

        # ---------------- SIM + softmax + readpass per batch ----------------
        if stop_phase >= 6:
          with contextlib.ExitStack() as ph:
              psim_pool = ph.enter_context(
                  tc.tile_pool(name="psim", bufs=1, space="PSUM"))
              pcs = ph.enter_context(tc.tile_pool(name="pcs", bufs=4, space="PSUM"))
              prd = ph.enter_context(tc.tile_pool(name="prd", bufs=1, space="PSUM"))
              pmt = ph.enter_context(tc.tile_pool(name="pmt", bufs=4))
              pewq = ph.enter_context(tc.tile_pool(name="pewq", bufs=3))

              def cross_sum(vec128, tag):
                  ps = pcs.tile([1, 1], FP, tag="cs")
                  nc.tensor.matmul(ps[:], vec128, ones128[:], start=True, stop=True)
                  sb = pewq.tile([1, 1], FP, tag=f"css_{tag}")
                  nc.vector.tensor_copy(out=sb[:], in_=ps[:])
                  return sb

              def bcast128(sc11, tag):
                  ps = pcs.tile([128, 1], FP, tag="cs")
                  nc.tensor.matmul(ps[:], ones1x128[:], sc11, start=True, stop=True)
                  sb = pewq.tile([128, 1], FP, tag=f"bcs_{tag}")
                  nc.vector.tensor_copy(out=sb[:], in_=ps[:])
                  return sb

              # ---- phase S0: normalized previous weights, all batches ----
              wpns = []
              for b in range(BC):
                  wpn_nat = pmt.tile([128, 128], FP, tag="wpn_nat")
                  nc.sync.dma_start(out=wpn_nat[:],
                                    in_=wprev_d[b].rearrange("(c p) -> c p", p=128))
                  wpT_ps = prd.tile([128, 128], FP, tag="trps")
                  nc.tensor.transpose(wpT_ps[:], wpn_nat[:], ident[:])
                  wpT = pewq.tile([128, NC128], FP, tag=f"wpT{b}")
                  nc.vector.tensor_copy(out=wpT[:], in_=wpT_ps[:])
                  wps = pewq.tile([128, 1], FP, tag="wps")
                  nc.vector.tensor_reduce(out=wps[:], in_=wpT[:], axis=AX.X,
                                          op=ALU.add)
                  wpt = cross_sum(wps[:], f"wpt{b}")
                  nc.vector.tensor_scalar_add(wpt[:], wpt[:], EPS)
                  wpr = pewq.tile([1, 1], FP, tag="wpr")
                  nc.vector.reciprocal(out=wpr[:], in_=wpt[:])
                  wpr128 = bcast128(wpr[:], "wpr")
                  wpn = pewq.tile([128, NC128], FP, tag=f"wpn{b}")
                  nc.vector.tensor_scalar_mul(wpn[:], wpT[:], wpr128[:])
                  wpns.append(wpn)

              # ---- phase S1: sim pass for ALL batches (keeps the DMA
              # transpose stream uninterrupted; psim pinned per batch) ----
              psims = []
              for b in range(BC):
                  psim = psim_pool.tile([128, NC128, 3], FP, tag=f"psim{b % 2}")
                  for c2 in range(N // 512):
                      mt = pmt.tile([128, 512], BF, tag="mt")
                      eng = nc.sync if c2 % 2 == 0 else nc.scalar
                      eng.dma_start_transpose(
                          mt[:], membf_d[b, c2 * 512:(c2 + 1) * 512, :])
                      sq = pmt.tile([128, 512], BF, tag="sq")
                      nc.vector.tensor_mul(out=sq[:], in0=mt[:], in1=mt[:])
                      for sub in range(4):
                          cc = c2 * 4 + sub
                          nc.tensor.matmul(psim[:, cc, 0:2],
                                           mt[:, sub * 128:(sub + 1) * 128],
                                           khl[:, b::BC], start=True, stop=True)
                          nc.tensor.matmul(psim[:, cc, 2:3],
                                           sq[:, sub * 128:(sub + 1) * 128],
                                           onesbf[:], start=True, stop=True)
                  psims.append(psim)

              # ---- phase S2: softmax chains for ALL batches (independent
              # DVE/ACT streams interleave), then read passes ----
              wv2s, ws128s = [], []
              for b in range(BC):
                  psim = psims[b]
                  wpn = wpns[b]
                  psb = pewq.tile([128, NC128, 3], FP, tag="psb")
                  nc.vector.tensor_copy(out=psb[:], in_=psim[:])
                  simd = pewq.tile([128, NC128], FP, tag="simd")
                  nc.vector.tensor_add(out=simd[:], in0=psb[:, :, 0],
                                       in1=psb[:, :, 1])
                  nrm = pewq.tile([128, NC128], FP, tag="nrm")
                  nc.scalar.activation(out=nrm[:], in_=psb[:, :, 2], func=AF.Sqrt)
                  nc.vector.tensor_scalar_add(nrm[:], nrm[:], EPS)
                  rec = pewq.tile([128, NC128], FP, tag="rec")
                  nc.vector.reciprocal(out=rec[:], in_=nrm[:])
                  bs = pewq.tile([128, NC128], FP, tag="bs")
                  nc.vector.tensor_mul(out=bs[:], in0=simd[:], in1=rec[:])
                  es = pewq.tile([128, NC128], FP, tag="es")
                  esum = pewq.tile([128, 1], FP, tag="esum")
                  nc.scalar.activation(out=es[:], in_=bs[:], func=AF.Exp,
                                       accum_out=esum[:])
                  etot = cross_sum(esum[:], "etot")
                  eret = pewq.tile([1, 1], FP, tag="eret")
                  nc.vector.reciprocal(out=eret[:], in_=etot[:])
                  er128 = bcast128(eret[:], "er")
                  wc = pewq.tile([128, NC128], FP, tag="wc")
                  nc.vector.tensor_scalar_mul(wc[:], es[:], er128[:])

                  gb = bcast128(gT[:, b:b + 1], "gb")
                  dwc = pewq.tile([128, NC128], FP, tag="dwc")
                  nc.vector.tensor_tensor(out=dwc[:], in0=wc[:], in1=wpn[:],
                                          op=ALU.subtract)
                  w0 = pewq.tile([128, NC128], FP, tag="w0")
                  nc.vector.scalar_tensor_tensor(out=w0[:], in0=dwc[:], scalar=gb[:],
                                                 in1=wpn[:], op0=ALU.mult,
                                                 op1=ALU.add)

                  gamb = bcast128(gamT[:, b:b + 1], "gamb")
                  lw = pewq.tile([128, NC128], FP, tag="lw")
                  nc.scalar.activation(out=lw[:], in_=w0[:], func=AF.Ln,
                                       bias=eps128[:])
                  wg = pewq.tile([128, NC128], FP, tag="wg")
                  wgs = pewq.tile([128, 1], FP, tag="wgs")
                  nc.scalar.activation(out=wg[:], in_=lw[:], func=AF.Exp,
                                       scale=gamb[:], accum_out=wgs[:])
                  wgt = cross_sum(wgs[:], "wgt")
                  wgr = pewq.tile([1, 1], FP, tag="wgr")
                  nc.vector.reciprocal(out=wgr[:], in_=wgt[:])
                  wgr128 = bcast128(wgr[:], "wgr")
                  wfin = pewq.tile([128, NC128], FP, tag="wfin")
                  nc.vector.tensor_scalar_mul(wfin[:], wg[:], wgr128[:])
                  if debug and b == 0:
                      nc.sync.dma_start(out=dbg_w_d[:], in_=wfin[:])

                  wsq = pewq.tile([128, NC128], FP, tag="wsq")
                  nc.vector.tensor_mul(out=wsq[:], in0=wfin[:], in1=wfin[:])
                  wss = pewq.tile([128, 1], FP, tag="wss")
                  nc.vector.tensor_reduce(out=wss[:], in_=wsq[:], axis=AX.X,
                                          op=ALU.add)
                  wst = cross_sum(wss[:], "wst")
                  ws128 = bcast128(wst[:], f"ws{b}")

                  wv2 = pewq.tile([128, NC128, 2], BF, tag=f"wv2{b}")
                  nc.vector.tensor_copy(out=wv2[:, :, 0], in_=wfin[:])
                  nc.vector.tensor_copy(out=wv2[:, :, 1], in_=wsq[:])
                  wv2s.append(wv2)
                  ws128s.append(ws128)

              for b in range(BC):
                  wv2, ws128 = wv2s[b], ws128s[b]
                  # read pass: out[2, 128] += wv2[:, cc, :].T @ membf[cc chunk]
                  pr2 = prd.tile([2, 128], FP, tag="pr2")
                  for c in range(N // 1024):
                      mn8 = pmt.tile([128, 8, 128], BF, tag="mn")
                      eng = nc.sync if c % 2 == 0 else nc.scalar
                      eng.dma_start(
                          out=mn8[:],
                          in_=membf_d[b, c * 1024:(c + 1) * 1024, :].rearrange(
                              "(s p) w -> p s w", p=128))
                      for sub in range(8):
                          cc = c * 8 + sub
                          nc.tensor.matmul(pr2[:], wv2[:, cc, :], mn8[:, sub, :],
                                           start=(cc == 0), stop=(cc == NC128 - 1))
                  pr2s = pewq.tile([2, 128], FP, tag="pr2s")
                  nc.vector.tensor_copy(out=pr2s[:], in_=pr2[:])
                  prT_ps = prd.tile([128, 2], FP, tag="trps")
                  nc.tensor.transpose(prT_ps[:], pr2s[:], ident[0:2, 0:2])
                  prT = pewq.tile([128, 2], FP, tag="prT")
                  nc.vector.tensor_copy(out=prT[:], in_=prT_ps[:])

                  u = pewq.tile([128, 1], FP, tag="u")
                  nc.vector.tensor_mul(out=u[:], in0=prT[:, 1:2], in1=eT[:, b:b + 1])
                  v = pewq.tile([128, 1], FP, tag="v")
                  nc.vector.tensor_tensor(out=v[:], in0=prT[:, 0:1], in1=u[:],
                                          op=ALU.subtract)
                  t5 = pewq.tile([128, 1], FP, tag="t5")
                  nc.vector.tensor_mul(out=t5[:], in0=aT[:, b:b + 1], in1=ws128[:])
                  rcol = pewq.tile([128, 1], FP, tag="rcol")
                  nc.vector.tensor_add(out=rcol[:], in0=v[:], in1=t5[:])
                  nc.vector.tensor_copy(out=combT[:, KH, b:b + 1], in_=rcol[:])

        # ---------------- phase OUT -----------------------------------------
        if stop_phase >= 7:
          with contextlib.ExitStack() as ph:
              pw = ph.enter_context(tc.tile_pool(name="pw_o", bufs=1))
              pps_o = ph.enter_context(tc.tile_pool(name="pps_o", bufs=1,
                                                    space="PSUM"))
              woutT = pw.tile([128, KH + 1, I], BF)
              for k in range(KH + 1):
                  nc.scalar.dma_start_transpose(
                      woutT[:, k, :], woutbf_d[:, k * 128:(k + 1) * 128])
              po = pps_o.tile([BC, I], FP, tag="po")
              for k in range(KH + 1):
                  nc.tensor.matmul(po[:], combT[:, k, :], woutT[:, k, :],
                                   start=(k == 0),
                                   stop=(k == KH and "bout" not in nzb))
              if "bout" in nzb:
                  bias_mm(po[:], bias_t["bout"][:], BC)
              ob = pw.tile([BC, I], FP, tag="ob")
              nc.vector.tensor_copy(out=ob[:], in_=po[:])
              nc.sync.dma_start(out=out_d[:], in_=ob[:])
              if debug:
                  rTdbg = pw.tile([128, BC], FP, tag="rTdbg")
                  nc.vector.tensor_copy(out=rTdbg[:], in_=combT[:, KH, :])
                  nc.sync.dma_start(out=dbg_r_d[:], in_=rTdbg[:])
        else:
            zo = ptmp.tile([BC, I], FP, tag='zo')
            nc.vector.memset(zo[:], 0.0)
            nc.sync.dma_start(out=out_d[:], in_=zo[:])
    nc.compile()
    return nc


_NC_CACHE = {}


def _get_nc(S, nzb_key, debug):
    sp = int(os.environ.get('BASSGRU_STOP', '7'))
    key = (S, nzb_key, debug, sp, os.environ.get("BASSGRU_SUB", "9"),
           os.environ.get("BASSGRU_SUBSTEP", "9"))
    if key not in _NC_CACHE:
        _NC_CACHE[key] = build_nc(S=S, nonzero_biases=nzb_key, debug=debug, stop_phase=sp)
    return _NC_CACHE[key]


def make_in_maps(inputs, S=S_FULL, debug=False):
    f32 = lambda a: np.ascontiguousarray(np.asarray(a), dtype=np.float32)
    bias_names = {"bih0": "b_ih0", "bhh0": "b_hh0", "bih1": "b_ih1",
                  "bhh1": "b_hh1", "bk": "bk", "bbeta": "bbeta", "bg": "bg",
                  "bgamma": "bgamma", "be": "be", "ba": "ba", "bout": "bout"}
    nzb = tuple(sorted(k for k, src in bias_names.items()
                       if np.any(np.asarray(inputs[src]) != 0)))
    nc = _get_nc(S, nzb, debug)
    shared = {nm: f32(inputs[nm]) for nm in
              ["W_ih0", "W_hh0", "W_ih1", "W_hh1", "Wk", "Wbeta", "Wg",
               "Wgamma", "We", "Wa", "Wout"]}
    for k, src in bias_names.items():
        if k in nzb:
            shared[k] = f32(inputs[src]).reshape(1, -1)
    x = f32(inputs["x"])
    mem = f32(inputs["memory"])
    wp = f32(inputs["w_prev"])
    in_maps = []
    for c in range(NCORES):
        m = dict(shared)
        m["x"] = x[c * BC:(c + 1) * BC]
        m["memory"] = mem[c * BC:(c + 1) * BC]
        m["w_prev"] = wp[c * BC:(c + 1) * BC]
        in_maps.append(m)
    return nc, in_maps, nzb


def kernel(**inputs) -> np.ndarray:
    debug = bool(int(os.environ.get("BASSGRU_DEBUG", "0")))
    S = int(os.environ.get("BASSGRU_S", str(S_FULL)))
    nc, in_maps, _ = make_in_maps(inputs, S=S, debug=debug)
    res = run_bass_kernel_spmd(nc, in_maps, list(range(NCORES)))
    outs = [res.results[c]["out"] for c in range(NCORES)]
    if debug:
        kernel.last_results = res.results
    return np.concatenate(outs, axis=0).astype(np.float32)


# revision 61
# speedup vs baseline: 1.0396x; 1.0396x over previous
import os
import sys
import contextlib
import numpy as np

sys.path.insert(0, "/opt/trn_rl_repo")

import concourse.bass as bass  # noqa: E402
import concourse.tile as tile  # noqa: E402
from concourse import bacc, mybir  # noqa: E402
from concourse.bass_utils import run_bass_kernel_spmd  # noqa: E402
from concourse.masks import make_identity  # noqa: E402

FP = mybir.dt.float32
BF = mybir.dt.bfloat16
FR = mybir.dt.float32r
AF = mybir.ActivationFunctionType
ALU = mybir.AluOpType
AX = mybir.AxisListType

# Problem dims (hardcoded; kernel.py must be self-contained)
B, S_FULL, I, H, N, W = 32, 128, 512, 1024, 16384, 128
TH = 3 * H          # 3072
NCORES = 8
BC = B // NCORES    # 4 batches per core
KH = H // 128       # 8
KI = I // 128       # 4
NT = TH // 512      # 6
NC128 = N // 128    # 128 chunks of memory rows
EPS = 1e-8


def _transpose_from_dram(nc, tc, pool_tmp, src_view, dst_tile, n_mchunks,
                         n_kchunks, ident, src_cols):
    """dst[p, k, m] = src[m, k*128+p]; src is [n_mchunks*128, src_cols] in
    DRAM, dst an SBUF tile [128, n_kchunks, n_mchunks*128] (any dtype; the
    PSUM->SBUF copy casts). Uses its own psum pool (2 banks), freed on
    return."""
    with tc.tile_pool(name="tr_ps", bufs=2, space="PSUM") as pps:
        for mj in range(n_mchunks):
            nat = pool_tmp.tile([128, src_cols], FP, tag="tr_nat")
            nc.sync.dma_start(out=nat[:], in_=src_view[mj * 128:(mj + 1) * 128, :])
            for ki in range(n_kchunks):
                tp = pps.tile([128, 128], FP, tag="tr_ps")
                nc.tensor.transpose(tp[:], nat[:, ki * 128:(ki + 1) * 128], ident[:])
                nc.vector.tensor_copy(
                    out=dst_tile[:, ki, mj * 128:(mj + 1) * 128], in_=tp[:])


def _emit_gru_step(nc, st, t):
    """One GRU step for layer state dict `st`.

    Fat layouts (partition = 32*g + b, only b<4 rows valid):
      - r,z gates: [128, 512], group g covers gate cols [512g, 512g+512)
        (g 0,1 = r over H; g 2,3 = z over H).
      - n gate / h state: [128, 256], group g covers H cols [256g, 256g+256).
    Col-tiled matmuls (tile_position=(0, 32g)) run the 4 groups concurrently
    on HW. gi arrives pre-scattered in the same fat layouts.
    """
    Tg = st["Tg"]
    if t % Tg == 0:
        gir = st["pgi"].tile([128, Tg, 256], BF, tag="gir")
        giz = st["pgi"].tile([128, Tg, 256], BF, tag="giz")
        gin = st["pgi"].tile([128, Tg, 256], BF, tag="gin")
        vR, vZ, vB = st["gi_view"](t)
        nc.sync.dma_start(out=gir[:], in_=vR)
        nc.sync.dma_start(out=giz[:], in_=vZ)
        nc.sync.dma_start(out=gin[:], in_=vB)
        st["gi_cur"] = (gir, giz, gin)
    giR = st["gi_cur"][0][:, t % Tg, :]
    giZ = st["gi_cur"][1][:, t % Tg, :]
    giN = st["gi_cur"][2][:, t % Tg, :]
    hT, h, whhT = st["hT"], st["h"], st["whhT"]
    pew = st["pew"]

    pgR = st["psR"].tile([128, 256], FP, tag="pgR")
    pgZ = st["psZ"].tile([128, 256], FP, tag="pgZ")
    pgN = st["psN"].tile([128, 256], FP, tag="pgN")
    # skip_group_check: the 4 col-groups run concurrent accumulation groups in
    # one psum bank at different base partitions; CoreSim's zero-region
    # tracker ignores base partition and would falsely flag a conflict (HW
    # has_written bits are per (partition, element) — verified on device).
    # hT is padded to 32 cols (zeros beyond BC) so every psum partition gets
    # written — downstream full-tile ops then read only defined data.
    for gate, pg in ((0, pgR), (1, pgZ), (2, pgN)):
        laststop = (st["bhh_n"] is None) or gate != 2
        for k in range(KH):
            for g in range(4):
                c0 = 1024 * gate + 256 * g
                nc.tensor.matmul(pg[32 * g:32 * g + 32, :], hT[:, k, :],
                                 whhT[:, k, c0:c0 + 256],
                                 start=(k == 0),
                                 stop=(k == KH - 1 and laststop),
                                 tile_position=(0, 32 * g),
                                 skip_group_check=True)
    if st["bhh_n"] is not None:
        for g in range(4):
            nc.tensor.matmul(pgN[32 * g:32 * g + 32, :], st["ones1x128"][:, 0:32],
                             st["bhh_n"][:, 256 * g:256 * (g + 1)],
                             start=False, stop=True, tile_position=(0, 32 * g),
                             skip_group_check=True)

    if st.get("substep", 9) < 2:
        return
    rp = pew.tile([128, 256], BF, tag="rp")
    nc.vector.tensor_add(out=rp[:], in0=pgR[:], in1=giR)
    r_f = pew.tile([128, 256], BF, tag="r_f")
    nc.scalar.activation(out=r_f[:], in_=rp[:], func=AF.Sigmoid)
    zp = pew.tile([128, 256], BF, tag="zp")
    nc.vector.tensor_add(out=zp[:], in0=pgZ[:], in1=giZ)
    z_f = pew.tile([128, 256], BF, tag="z_f")
    nc.scalar.activation(out=z_f[:], in_=zp[:], func=AF.Sigmoid)

    npf = pew.tile([128, 256], BF, tag="npf")
    nc.vector.tensor_mul(out=npf[:], in0=pgN[:], in1=r_f[:])
    nf = pew.tile([128, 256], BF, tag="nf")
    nc.vector.tensor_add(out=nf[:], in0=npf[:], in1=giN)
    n_f = pew.tile([128, 256], BF, tag="n_f")
    nc.scalar.activation(out=n_f[:], in_=nf[:], func=AF.Tanh)

    hmn = pew.tile([128, 256], BF, tag="hmn")
    nc.vector.tensor_tensor(out=hmn[:], in0=h[:], in1=n_f[:], op=ALU.subtract)
    h2 = st["phh"].tile([128, 256], BF, tag="h")
    nc.vector.tensor_mul(out=h2[:], in0=hmn[:], in1=z_f[:])
    nc.vector.tensor_add(out=h2[:], in0=h2[:], in1=n_f[:])

    st["h"] = h2
    st["h2_pend"] = h2


def _emit_gru_step_tail(nc, st, t):
    """Transpose h2 for the next step's stationary. Emitted after the other
    layer's matmuls so the PE isn't stalled waiting on this layer's chain."""
    if st.get("substep", 9) < 3 or "h2_pend" not in st:
        return
    h2 = st.pop("h2_pend")
    pew = st["pew"]
    # fat -> natural copy; row-positioned transposes straight off the fat
    # layout read garbage on HW when mixed with col-tiled matmuls, so keep
    # all transposes at base partition 0
    h2n = pew.tile([BC, 1024], BF, tag="h2n")
    for q in range(4):
        nc.vector.tensor_copy(out=h2n[:, 256 * q:256 * (q + 1)],
                              in_=h2[32 * q:32 * q + 4, :])
    htp = st["pht"].tile([128, KH, BC], BF, tag="htp")
    for j in range(KH):
        nc.tensor.transpose(htp[:, j, :], h2n[:, 128 * j:128 * (j + 1)],
                            st["identbf"][0:BC, 0:BC])
    hT2 = st["hT_slots"][(t + 1) % 2]
    nc.vector.tensor_copy(out=hT2[:, :, 0:BC], in_=htp[:])
    if st.get("y0blk") is not None and st.get("substep", 9) >= 5:
        nc.vector.tensor_copy(out=st["y0blk"][:, :, :, t % st["TBLK"]],
                              in_=htp[:])
    if st.get("substep", 9) >= 4:
        st["hT"] = hT2


def build_nc(S=S_FULL, nonzero_biases=(), debug=False, stop_phase=7):
    nzb = set(nonzero_biases)
    nc = bacc.Bacc("TRN2", target_bir_lowering=False, debug=False,
                   num_devices=NCORES)
    Tg = min(4, S)
    TBLK = min(16, S)
    assert S % TBLK == 0 and TBLK % Tg == 0

    # ---- DRAM I/O ----
    x_d = nc.declare_dram_parameter("x", [BC, S_FULL, I], FP, isOutput=False)
    mem_d = nc.declare_dram_parameter("memory", [BC, N, W], FP, isOutput=False)
    wprev_d = nc.declare_dram_parameter("w_prev", [BC, N], FP, isOutput=False)
    Wih0_d = nc.declare_dram_parameter("W_ih0", [TH, I], FP, isOutput=False)
    Whh0_d = nc.declare_dram_parameter("W_hh0", [TH, H], FP, isOutput=False)
    Wih1_d = nc.declare_dram_parameter("W_ih1", [TH, H], FP, isOutput=False)
    Whh1_d = nc.declare_dram_parameter("W_hh1", [TH, H], FP, isOutput=False)
    Wk_d = nc.declare_dram_parameter("Wk", [H, W], FP, isOutput=False)
    Wbeta_d = nc.declare_dram_parameter("Wbeta", [H, 1], FP, isOutput=False)
    Wg_d = nc.declare_dram_parameter("Wg", [H, 1], FP, isOutput=False)
    Wgamma_d = nc.declare_dram_parameter("Wgamma", [H, 1], FP, isOutput=False)
    We_d = nc.declare_dram_parameter("We", [H, W], FP, isOutput=False)
    Wa_d = nc.declare_dram_parameter("Wa", [H, W], FP, isOutput=False)
    Wout_d = nc.declare_dram_parameter("Wout", [I, H + W], FP, isOutput=False)
    bias_d = {}
    for nm, sz in [("bih0", TH), ("bhh0", TH), ("bih1", TH), ("bhh1", TH),
                   ("bk", W), ("bbeta", 1), ("bg", 1), ("bgamma", 1),
                   ("be", W), ("ba", W), ("bout", I)]:
        if nm in nzb:
            bias_d[nm] = nc.declare_dram_parameter(nm, [1, sz], FP,
                                                   isOutput=False)
    out_d = nc.declare_dram_parameter("out", [BC, I], FP, isOutput=True)
    if debug:
        dbg_h1_d = nc.declare_dram_parameter("dbg_h1", [BC, H], FP, isOutput=True)
        dbg_head_d = nc.declare_dram_parameter("dbg_head", [BC, 387], FP,
                                               isOutput=True)
        dbg_w_d = nc.declare_dram_parameter("dbg_w", [128, NC128], FP,
                                            isOutput=True)
        dbg_r_d = nc.declare_dram_parameter("dbg_r", [128, BC], FP, isOutput=True)

    # gi scratch in "fat" layout: [t, g, slot32, 256] per gate (r/z/n) so a
    # step's gate rows land at partitions 32*g+b directly (slots 4..31 unused).
    gi0_slabs = [[nc.dram_tensor(f"gi0{gn}_scr{j}", [32, 4, 32, 256], BF)
                  for gn in "rzn"] for j in range(S_FULL // 32)]
    gi1_gates = [[nc.dram_tensor(f"gi1{gn}_scr{i}", [TBLK, 4, 32, 256], BF)
                  for gn in "rzn"] for i in range(S // TBLK)]
    membf_d = nc.dram_tensor("membf_scr", [BC, N, W], BF)
    xbf_d = nc.dram_tensor("xbf_scr", [BC * S_FULL, I], BF)
    wih0bf_d = nc.dram_tensor("wih0bf_scr", [TH, I], BF)
    woutbf_d = nc.dram_tensor("woutbf_scr", [I, H + W], BF)
    whh0bf_d = nc.dram_tensor("whh0bf_scr", [TH, H], BF)
    whh1bf_d = nc.dram_tensor("whh1bf_scr", [TH, H], BF)
    wih1bf_d = nc.dram_tensor("wih1bf_scr", [TH, H], BF)

    with tile.TileContext(nc) as tc, contextlib.ExitStack() as top:
        const = top.enter_context(tc.tile_pool(name="const", bufs=1))
        ptmp = top.enter_context(tc.tile_pool(name="ptmp", bufs=2))

        ident = const.tile([128, 128], FP)
        make_identity(nc, ident[:])
        ones1x128 = const.tile([1, 128], FP)
        nc.vector.memset(ones1x128[:], 1.0)
        ones128 = const.tile([128, 1], FP)
        nc.vector.memset(ones128[:], 1.0)
        onesbf = const.tile([128, 1], BF)
        nc.vector.memset(onesbf[:], 1.0)
        eps128 = const.tile([128, 1], FP)
        nc.vector.memset(eps128[:], EPS)
        ones1bc = const.tile([1, BC], FP)
        nc.vector.memset(ones1bc[:], 1.0)
        identbf = const.tile([128, 128], BF)
        nc.vector.tensor_copy(out=identbf[:], in_=ident[:])

        # early DRAM->DRAM casts (gpsimd queue; overlap with A0/GRU)
        nc.gpsimd.dma_start(
            out=xbf_d[:].rearrange("(s b) i -> s b i", b=BC),
            in_=x_d[:].rearrange("b s i -> s b i"))
        nc.gpsimd.dma_start(out=wih0bf_d[:], in_=Wih0_d[:])
        nc.gpsimd.dma_start(out=whh0bf_d[:], in_=Whh0_d[:])
        nc.gpsimd.dma_start(out=whh1bf_d[:], in_=Whh1_d[:])
        nc.gpsimd.dma_start(out=wih1bf_d[:], in_=Wih1_d[:])
        nc.gpsimd.dma_start(out=woutbf_d[:], in_=Wout_d[:])
        for b in range(BC):
            nc.gpsimd.dma_start(out=membf_d[b], in_=mem_d[b])

        bias_t = {}
        for nm in bias_d:
            t = const.tile([1, bias_d[nm].shape[1]], FP, tag=f"b_{nm}")
            nc.sync.dma_start(out=t[:], in_=bias_d[nm][:])
            bias_t[nm] = t

        def bias_mm(psum_ap, src_ap, nrows):
            nc.tensor.matmul(psum_ap, ones1x128[:, 0:nrows], src_ap,
                             start=False, stop=True)

        # ---------------- phase A0: gi0 = x @ W_ih0.T (+ biases) -------------
        with contextlib.ExitStack() as ph:
            pw = ph.enter_context(tc.tile_pool(name="pw_a0", bufs=1))
            ptb = ph.enter_context(tc.tile_pool(name="ptmp_a0", bufs=2))
            xT = pw.tile([128, KI, BC * S_FULL], BF)
            wT = pw.tile([128, KI, TH], BF)
            for ki in range(KI):
                nc.scalar.dma_start_transpose(
                    xT[:, ki, :], xbf_d[:, ki * 128:(ki + 1) * 128])
                nc.scalar.dma_start_transpose(
                    wT[:, ki, :], wih0bf_d[:, ki * 128:(ki + 1) * 128])

            bsum = None
            if "bih0" in nzb or "bhh0" in nzb:
                bsum = pw.tile([1, TH], FP, tag="bsum0")
                nc.vector.memset(bsum[:], 0.0)
                if "bih0" in nzb:
                    nc.vector.tensor_copy(out=bsum[:], in_=bias_t["bih0"][:])
                if "bhh0" in nzb:
                    nc.vector.tensor_add(out=bsum[:, 0:2048],
                                         in0=bsum[:, 0:2048],
                                         in1=bias_t["bhh0"][:, 0:2048])

            with tc.tile_pool(name="pps_a0", bufs=1, space="PSUM") as pps:
                for rj in range(BC * S_FULL // 128):
                    pg = pps.tile([128, NT, 512], FP, tag="pg_a0")
                    for nt in range(NT):
                        for ki in range(KI):
                            nc.tensor.matmul(
                                pg[:, nt, :], xT[:, ki, rj * 128:(rj + 1) * 128],
                                wT[:, ki, nt * 512:(nt + 1) * 512],
                                start=(ki == 0),
                                stop=(ki == KI - 1 and bsum is None))
                        if bsum is not None:
                            bias_mm(pg[:, nt, :],
                                    bsum[:, nt * 512:(nt + 1) * 512], 128)
                    gs = ptb.tile([128, TH], BF, tag="gs_a0")
                    nc.vector.tensor_copy(out=gs[:],
                                          in_=pg[:].rearrange("p n x -> p (n x)"))
                    # x rows are (s b)-major: chunk rj = steps [32rj, 32rj+32)
                    # for all b; write one gi0 slab so L0 can start after the
                    # first chunk instead of after all of A0
                    for gate in range(3):
                        for g in range(4):
                            c0 = 1024 * gate + 256 * g
                            nc.sync.dma_start(
                                out=gi0_slabs[rj][gate][:, g, 0:BC, :],
                                in_=gs[:, c0:c0 + 256])

        # ------------- interleaved recurrences L0 + L1 -----------------------
        pkeep = top.enter_context(tc.tile_pool(name="pkeep", bufs=1))
        h1T_keep = pkeep.tile([128, KH, BC], BF)
        h1_keep = pkeep.tile([BC, H], FP)
        if stop_phase >= 2:
          with contextlib.ExitStack() as ph:
            pw = ph.enter_context(tc.tile_pool(name="pw_gru", bufs=1))
            whh0T = pw.tile([128, KH, TH], BF, tag="whh0T")
            whh1T = pw.tile([128, KH, TH], BF, tag="whh1T")
            wih1T = pw.tile([128, KH, TH], BF, tag="wih1T")
            # layer-0 weights first: they gate the start of the recurrence
            for k in range(KH):
                nc.scalar.dma_start_transpose(
                    whh0T[:, k, :], whh0bf_d[:, k * 128:(k + 1) * 128])
            for k in range(KH):
                nc.scalar.dma_start_transpose(
                    wih1T[:, k, :], wih1bf_d[:, k * 128:(k + 1) * 128])
                nc.scalar.dma_start_transpose(
                    whh1T[:, k, :], whh1bf_d[:, k * 128:(k + 1) * 128])

            bsum1 = None
            if "bih1" in nzb or "bhh1" in nzb:
                bsum1 = pw.tile([1, TH], FP, tag="bsum1")
                nc.vector.memset(bsum1[:], 0.0)
                if "bih1" in nzb:
                    nc.vector.tensor_copy(out=bsum1[:], in_=bias_t["bih1"][:])
                if "bhh1" in nzb:
                    nc.vector.tensor_add(out=bsum1[:, 0:2048],
                                         in0=bsum1[:, 0:2048],
                                         in1=bias_t["bhh1"][:, 0:2048])
            bhh0n = bhh1n = None
            if "bhh0" in nzb:
                bhh0n = pw.tile([1, 1024], FP, tag="bhh0n")
                nc.vector.tensor_copy(out=bhh0n[:], in_=bias_t["bhh0"][:, 2048:TH])
            if "bhh1" in nzb:
                bhh1n = pw.tile([1, 1024], FP, tag="bhh1n")
                nc.vector.tensor_copy(out=bhh1n[:], in_=bias_t["bhh1"][:, 2048:TH])

            py0 = ph.enter_context(tc.tile_pool(name="py0", bufs=2))
            psG = ph.enter_context(tc.tile_pool(name="psG", bufs=1, space="PSUM"))
            pgo = ph.enter_context(tc.tile_pool(name="pgo", bufs=2))

            def gi0_view(t):
                j, lo = t // 32, t % 32
                return tuple(d[lo:lo + Tg].rearrange("t g q x -> (g q) t x")
                             for d in gi0_slabs[j])

            def gi1_view(t):
                blk, lo = t // TBLK, t % TBLK
                return tuple(d[lo:lo + Tg].rearrange("t g q x -> (g q) t x")
                             for d in gi1_gates[blk])

            pht_shared = ph.enter_context(
                tc.tile_pool(name="pht", bufs=1, space="PSUM"))
            sts = []
            for li, (whhT, gi_view, bhh_n) in enumerate(
                    [(whh0T, gi0_view, bhh0n), (whh1T, gi1_view, bhh1n)]):
                st = {
                    "Tg": Tg, "TBLK": TBLK, "gi_view": gi_view, "whhT": whhT,
                    "bhh_n": bhh_n, "ident": ident, "identbf": identbf,
                    "ones1bc": ones1bc,
                    "psR": ph.enter_context(
                        tc.tile_pool(name=f"psR{li}", bufs=1, space="PSUM")),
                    "psZ": ph.enter_context(
                        tc.tile_pool(name=f"psZ{li}", bufs=1, space="PSUM")),
                    "psN": ph.enter_context(
                        tc.tile_pool(name=f"psN{li}", bufs=1, space="PSUM")),
                    "pht": pht_shared,
                    "pew": ph.enter_context(tc.tile_pool(name=f"pew{li}", bufs=1)),
                    "pgi": ph.enter_context(tc.tile_pool(name=f"pgi{li}", bufs=2)),
                    "phh": ph.enter_context(tc.tile_pool(name=f"phh{li}", bufs=2)),
                }
                st["ones1x128"] = ones1x128
                st["substep"] = int(os.environ.get("BASSGRU_SUBSTEP", "9"))
                h = st["phh"].tile([128, 256], BF, tag="h")
                nc.vector.memset(h[:], 0.0)
                # persistent double-buffered transposed-h slots, padded to 32
                # cols; the zero padding makes the matmuls write every psum
                # partition with defined values
                slots = []
                for sl in range(2):
                    hTs = pw.tile([128, KH, 32], BF, tag=f"hTs{li}_{sl}")
                    nc.vector.memset(hTs[:], 0.0)
                    slots.append(hTs)
                st["hT_slots"] = slots
                st["h"], st["hT"] = h, slots[0]
                sts.append(st)

            def emit_gi1_block(blk, y0blk):
                BT = BC * TBLK
                for nt in range(NT):
                    pg = psG.tile([BT, 512], FP, tag="pgG")
                    for k in range(KH):
                        nc.tensor.matmul(pg[:], y0blk[:, k, :, :],
                                         wih1T[:, k, 512 * nt:512 * (nt + 1)],
                                         start=(k == 0),
                                         stop=(k == KH - 1 and bsum1 is None))
                    if bsum1 is not None:
                        bias_mm(pg[:], bsum1[:, nt * 512:(nt + 1) * 512], BT)
                    gs = pgo.tile([BT, 512], BF, tag="gsG")
                    nc.vector.tensor_copy(out=gs[:], in_=pg[:])
                    gate, gbase = nt // 2, 2 * (nt % 2)
                    for half in range(2):
                        nc.sync.dma_start(
                            out=gi1_gates[blk][gate][:, gbase + half, 0:BC, :]
                            .rearrange("t b x -> b t x"),
                            in_=gs[:, 256 * half:256 * (half + 1)])

            sub = int(os.environ.get("BASSGRU_SUB", "9"))
            y0blk = None
            for t in range(S):
                if t % TBLK == 0:
                    y0blk = py0.tile([128, KH, BC, TBLK], BF, tag="y0blk")
                    sts[0]["y0blk"] = y0blk
                if sub >= 2:
                    _emit_gru_step(nc, sts[0], t)
                if stop_phase >= 3 and t >= TBLK:
                    _emit_gru_step(nc, sts[1], t - TBLK)
                if sub >= 2:
                    _emit_gru_step_tail(nc, sts[0], t)
                if stop_phase >= 3 and t >= TBLK:
                    _emit_gru_step_tail(nc, sts[1], t - TBLK)
                if (t + 1) % TBLK == 0 and sub >= 3:
                    emit_gi1_block(t // TBLK, y0blk)
            if stop_phase >= 3:
                for t in range(S - TBLK, S):
                    _emit_gru_step(nc, sts[1], t)
                    _emit_gru_step_tail(nc, sts[1], t)
                nc.vector.tensor_copy(out=h1T_keep[:],
                                      in_=sts[1]["hT"][:, :, 0:BC])
                for q in range(4):
                    nc.vector.tensor_copy(
                        out=h1_keep[:, 256 * q:256 * (q + 1)],
                        in_=sts[1]["h"][32 * q:32 * q + 4, :])
                if debug:
                    nc.sync.dma_start(out=dbg_h1_d[:], in_=h1_keep[:])

        # ---------------- phase H: NTM head ---------------------------------
        if stop_phase >= 5:
          hp = top.enter_context(tc.tile_pool(name="hp", bufs=1))
          ph_psum_stack = contextlib.ExitStack()
          pps_h = ph_psum_stack.enter_context(
              tc.tile_pool(name="pps_h", bufs=2, space="PSUM"))

          wcatf = hp.tile([128, KH, 512], FP, tag="wcatf")
          nc.vector.memset(wcatf[:], 0.0)
          nc.sync.dma_start(out=wcatf[:, :, 0:128],
                            in_=Wk_d[:].rearrange("(k p) w -> p k w", p=128))
          nc.sync.dma_start(out=wcatf[:, :, 128:256],
                            in_=We_d[:].rearrange("(k p) w -> p k w", p=128))
          nc.sync.dma_start(out=wcatf[:, :, 256:384],
                            in_=Wa_d[:].rearrange("(k p) w -> p k w", p=128))
          nc.sync.dma_start(out=wcatf[:, :, 384:385],
                            in_=Wbeta_d[:].rearrange("(k p) w -> p k w", p=128))
          nc.sync.dma_start(out=wcatf[:, :, 385:386],
                            in_=Wg_d[:].rearrange("(k p) w -> p k w", p=128))
          nc.sync.dma_start(out=wcatf[:, :, 386:387],
                            in_=Wgamma_d[:].rearrange("(k p) w -> p k w", p=128))
          wcat = hp.tile([128, KH, 512], BF, tag="wcat")
          nc.vector.tensor_copy(out=wcat[:], in_=wcatf[:])

          bcat = None
          if any(nm in nzb for nm in ("bk", "bbeta", "bg", "bgamma", "be", "ba")):
              bcat = hp.tile([1, 512], FP, tag="bcat")
              nc.vector.memset(bcat[:], 0.0)
              for nm, lo, hi in [("bk", 0, 128), ("be", 128, 256), ("ba", 256, 384),
                                 ("bbeta", 384, 385), ("bg", 385, 386),
                                 ("bgamma", 386, 387)]:
                  if nm in nzb:
                      nc.vector.tensor_copy(out=bcat[:, lo:hi], in_=bias_t[nm][:])

          phead = pps_h.tile([BC, 512], FP, tag="hps")
          for k in range(KH):
              nc.tensor.matmul(phead[:], h1T_keep[:, k, :], wcat[:, k, :],
                               start=(k == 0),
                               stop=(k == KH - 1 and bcat is None))
          if bcat is not None:
              bias_mm(phead[:], bcat[:], BC)
          head = hp.tile([BC, 512], FP, tag="head")
          nc.vector.tensor_copy(out=head[:], in_=phead[:])
          if debug:
              nc.sync.dma_start(out=dbg_head_d[:], in_=head[:, 0:387])

          e_t = hp.tile([BC, 128], FP, tag="e_t")
          nc.scalar.activation(out=e_t[:], in_=head[:, 128:256], func=AF.Sigmoid)
          a_t = hp.tile([BC, 128], FP, tag="a_t")
          nc.scalar.activation(out=a_t[:], in_=head[:, 256:384], func=AF.Tanh)
          # softplus(x) = ln(1 + exp(x)) for beta and gamma (no Softplus table)
          bg2 = hp.tile([BC, 2], FP, tag="bg2")
          nc.scalar.activation(out=bg2[:, 0:1], in_=head[:, 384:385], func=AF.Exp)
          nc.scalar.activation(out=bg2[:, 1:2], in_=head[:, 386:387], func=AF.Exp)
          nc.vector.tensor_scalar_add(bg2[:], bg2[:], 1.0)
          bg2l = hp.tile([BC, 2], FP, tag="bg2l")
          nc.scalar.activation(out=bg2l[:], in_=bg2[:], func=AF.Ln)
          beta_t = hp.tile([BC, 1], FP, tag="beta_t")
          nc.vector.tensor_copy(out=beta_t[:], in_=bg2l[:, 0:1])
          g_t = hp.tile([BC, 1], FP, tag="g_t")
          nc.scalar.activation(out=g_t[:], in_=head[:, 385:386], func=AF.Sigmoid)
          gam_t = hp.tile([BC, 1], FP, tag="gam_t")
          nc.vector.tensor_scalar_add(gam_t[:], bg2l[:, 1:2], 1.0)

          k_t = hp.tile([BC, 128], FP, tag="k_t")
          nc.vector.tensor_copy(out=k_t[:], in_=head[:, 0:128])
          kn2 = hp.tile([BC, 1], FP, tag="kn2")
          ksc = hp.tile([BC, 128], FP, tag="ksc")
          nc.vector.tensor_mul(out=ksc[:], in0=k_t[:], in1=k_t[:])
          nc.vector.tensor_reduce(out=kn2[:], in_=ksc[:], axis=AX.X,
                                  op=ALU.add)
          knrm = hp.tile([BC, 1], FP, tag="knrm")
          nc.scalar.activation(out=knrm[:], in_=kn2[:], func=AF.Sqrt)
          nc.vector.tensor_scalar_add(knrm[:], knrm[:], EPS)
          krec = hp.tile([BC, 1], FP, tag="krec")
          nc.vector.reciprocal(out=krec[:], in_=knrm[:])
          nc.vector.tensor_scalar_mul(krec[:], krec[:], beta_t[:])
          kb = hp.tile([BC, 128], FP, tag="kb")
          nc.vector.tensor_scalar_mul(kb[:], k_t[:], krec[:])

          def tr_small(src_ap, nrows, ncols, tag):
              tp = pps_h.tile([ncols, nrows], FP, tag="hps_tr")
              nc.tensor.transpose(tp[:], src_ap, ident[0:nrows, 0:nrows])
              dst = hp.tile([ncols, nrows], FP, tag=tag)
              nc.vector.tensor_copy(out=dst[:], in_=tp[:])
              return dst

          kbT = tr_small(kb[:], BC, 128, "kbT")
          eT = tr_small(e_t[:], BC, 128, "eT")
          aT = tr_small(a_t[:], BC, 128, "aT")
          gT = tr_small(g_t[:], BC, 1, "gT")
          gamT = tr_small(gam_t[:], BC, 1, "gamT")

          khl = hp.tile([128, 2 * BC], BF, tag="khl")
          nc.vector.tensor_copy(out=khl[:, 0:BC], in_=kbT[:])
          klo = hp.tile([128, BC], FP, tag="klo")
          nc.vector.tensor_tensor(out=klo[:], in0=kbT[:], in1=khl[:, 0:BC],
                                  op=ALU.subtract)
          nc.vector.tensor_copy(out=khl[:, BC:2 * BC], in_=klo[:])

          combT = pkeep.tile([128, KH + 1, BC], BF, tag="combT")
          nc.vector.tensor_copy(out=combT[:, 0:KH, :], in_=h1T_keep[:])

          ph_psum_stack.close()

        # ---------------- SIM + softmax + readpass per batch ----------------
        if stop_phase >= 6:
          with contextlib.ExitStack() as ph:
              psim_pool = ph.enter_context(
                  tc.tile_pool(name="psim", bufs=1, space="PSUM"))
              pcs = ph.enter_context(tc.tile_pool(name="pcs", bufs=4, space="PSUM"))
              prd = ph.enter_context(tc.tile_pool(name="prd", bufs=1, space="PSUM"))
              pmt = ph.enter_context(tc.tile_pool(name="pmt", bufs=8))
              pewq = ph.enter_context(tc.tile_pool(name="pewq", bufs=3))

              def cross_sum(vec128, tag):
                  ps = pcs.tile([1, 1], FP, tag="cs")
                  nc.tensor.matmul(ps[:], vec128, ones128[:], start=True, stop=True)
                  sb = pewq.tile([1, 1], FP, tag=f"css_{tag}")
                  nc.vector.tensor_copy(out=sb[:], in_=ps[:])
                  return sb

              def bcast128(sc11, tag):
                  ps = pcs.tile([128, 1], FP, tag="cs")
                  nc.tensor.matmul(ps[:], ones1x128[:], sc11, start=True, stop=True)
                  sb = pewq.tile([128, 1], FP, tag=f"bcs_{tag}")
                  nc.vector.tensor_copy(out=sb[:], in_=ps[:])
                  return sb

              # ---- phase S0: normalized previous weights, all batches ----
              wpns = []
              for b in range(BC):
                  wpn_nat = pmt.tile([128, 128], FP, tag="wpn_nat")
                  nc.sync.dma_start(out=wpn_nat[:],
                                    in_=wprev_d[b].rearrange("(c p) -> c p", p=128))
                  wpT_ps = prd.tile([128, 128], FP, tag="trps")
                  nc.tensor.transpose(wpT_ps[:], wpn_nat[:], ident[:])
                  wpT = pewq.tile([128, NC128], FP, tag=f"wpT{b}")
                  nc.vector.tensor_copy(out=wpT[:], in_=wpT_ps[:])
                  wps = pewq.tile([128, 1], FP, tag="wps")
                  nc.vector.tensor_reduce(out=wps[:], in_=wpT[:], axis=AX.X,
                                          op=ALU.add)
                  wpt = cross_sum(wps[:], f"wpt{b}")
                  nc.vector.tensor_scalar_add(wpt[:], wpt[:], EPS)
                  wpr = pewq.tile([1, 1], FP, tag="wpr")
                  nc.vector.reciprocal(out=wpr[:], in_=wpt[:])
                  wpr128 = bcast128(wpr[:], "wpr")
                  wpn = pewq.tile([128, NC128], FP, tag=f"wpn{b}")
                  nc.vector.tensor_scalar_mul(wpn[:], wpT[:], wpr128[:])
                  wpns.append(wpn)

              # ---- phase S1: sim pass for ALL batches (keeps the DMA
              # transpose stream uninterrupted; psim pinned per batch) ----
              psims = []
              for b in range(BC):
                  psim = psim_pool.tile([128, NC128, 3], FP, tag=f"psim{b % 2}")
                  for c2 in range(N // 512):
                      mt = pmt.tile([128, 512], BF, tag="mt")
                      eng = nc.sync if c2 % 2 == 0 else nc.scalar
                      eng.dma_start_transpose(
                          mt[:], membf_d[b, c2 * 512:(c2 + 1) * 512, :])
                      sq = pmt.tile([128, 512], BF, tag="sq")
                      nc.vector.tensor_mul(out=sq[:], in0=mt[:], in1=mt[:])
                      for sub in range(4):
                          cc = c2 * 4 + sub
                          nc.tensor.matmul(psim[:, cc, 0:2],
                                           mt[:, sub * 128:(sub + 1) * 128],
                                           khl[:, b::BC], start=True, stop=True)
                          nc.tensor.matmul(psim[:, cc, 2:3],
                                           sq[:, sub * 128:(sub + 1) * 128],
                                           onesbf[:], start=True, stop=True)
                  psims.append(psim)

              # ---- phase S2: softmax chains for ALL batches (independent
              # DVE/ACT streams interleave), then read passes ----
              wv2s, ws128s = [], []
              for b in range(BC):
                  psim = psims[b]
                  wpn = wpns[b]
                  psb = pewq.tile([128, NC128, 3], FP, tag="psb")
                  nc.vector.tensor_copy(out=psb[:], in_=psim[:])
                  simd = pewq.tile([128, NC128], FP, tag="simd")
                  nc.vector.tensor_add(out=simd[:], in0=psb[:, :, 0],
                                       in1=psb[:, :, 1])
                  nrm = pewq.tile([128, NC128], FP, tag="nrm")
                  nc.scalar.activation(out=nrm[:], in_=psb[:, :, 2], func=AF.Sqrt)
                  nc.vector.tensor_scalar_add(nrm[:], nrm[:], EPS)
                  rec = pewq.tile([128, NC128], FP, tag="rec")
                  nc.vector.reciprocal(out=rec[:], in_=nrm[:])
                  bs = pewq.tile([128, NC128], FP, tag="bs")
                  nc.vector.tensor_mul(out=bs[:], in0=simd[:], in1=rec[:])
                  es = pewq.tile([128, NC128], FP, tag="es")
                  esum = pewq.tile([128, 1], FP, tag="esum")
                  nc.scalar.activation(out=es[:], in_=bs[:], func=AF.Exp,
                                       accum_out=esum[:])
                  etot = cross_sum(esum[:], "etot")
                  eret = pewq.tile([1, 1], FP, tag="eret")
                  nc.vector.reciprocal(out=eret[:], in_=etot[:])
                  er128 = bcast128(eret[:], "er")
                  wc = pewq.tile([128, NC128], FP, tag="wc")
                  nc.vector.tensor_scalar_mul(wc[:], es[:], er128[:])

                  gb = bcast128(gT[:, b:b + 1], "gb")
                  dwc = pewq.tile([128, NC128], FP, tag="dwc")
                  nc.vector.tensor_tensor(out=dwc[:], in0=wc[:], in1=wpn[:],
                                          op=ALU.subtract)
                  w0 = pewq.tile([128, NC128], FP, tag="w0")
                  nc.vector.scalar_tensor_tensor(out=w0[:], in0=dwc[:], scalar=gb[:],
                                                 in1=wpn[:], op0=ALU.mult,
                                                 op1=ALU.add)

                  gamb = bcast128(gamT[:, b:b + 1], "gamb")
                  lw = pewq.tile([128, NC128], FP, tag="lw")
                  nc.scalar.activation(out=lw[:], in_=w0[:], func=AF.Ln,
                                       bias=eps128[:])
                  wg = pewq.tile([128, NC128], FP, tag="wg")
                  wgs = pewq.tile([128, 1], FP, tag="wgs")
                  nc.scalar.activation(out=wg[:], in_=lw[:], func=AF.Exp,
                                       scale=gamb[:], accum_out=wgs[:])
                  wgt = cross_sum(wgs[:], "wgt")
                  wgr = pewq.tile([1, 1], FP, tag="wgr")
                  nc.vector.reciprocal(out=wgr[:], in_=wgt[:])
                  wgr128 = bcast128(wgr[:], "wgr")
                  wfin = pewq.tile([128, NC128], FP, tag="wfin")
                  nc.vector.tensor_scalar_mul(wfin[:], wg[:], wgr128[:])
                  if debug and b == 0:
                      nc.sync.dma_start(out=dbg_w_d[:], in_=wfin[:])

                  wsq = pewq.tile([128, NC128], FP, tag="wsq")
                  nc.vector.tensor_mul(out=wsq[:], in0=wfin[:], in1=wfin[:])
                  wss = pewq.tile([128, 1], FP, tag="wss")
                  nc.vector.tensor_reduce(out=wss[:], in_=wsq[:], axis=AX.X,
                                          op=ALU.add)
                  wst = cross_sum(wss[:], "wst")
                  ws128 = bcast128(wst[:], f"ws{b}")

                  wv2 = pewq.tile([128, NC128, 2], BF, tag=f"wv2{b}")
                  nc.vector.tensor_copy(out=wv2[:, :, 0], in_=wfin[:])
                  nc.vector.tensor_copy(out=wv2[:, :, 1], in_=wsq[:])
                  wv2s.append(wv2)
                  ws128s.append(ws128)

              for b in range(BC):
                  wv2, ws128 = wv2s[b], ws128s[b]
                  # read pass: out[2, 128] += wv2[:, cc, :].T @ membf[cc chunk]
                  pr2 = prd.tile([2, 128], FP, tag="pr2")
                  for c in range(N // 1024):
                      mn8 = pmt.tile([128, 8, 128], BF, tag="mn")
                      eng = nc.sync if c % 2 == 0 else nc.scalar
                      eng.dma_start(
                          out=mn8[:],
                          in_=membf_d[b, c * 1024:(c + 1) * 1024, :].rearrange(
                              "(s p) w -> p s w", p=128))
                      for sub in range(8):
                          cc = c * 8 + sub
                          nc.tensor.matmul(pr2[:], wv2[:, cc, :], mn8[:, sub, :],
                                           start=(cc == 0), stop=(cc == NC128 - 1))
                  pr2s = pewq.tile([2, 128], FP, tag="pr2s")
                  nc.vector.tensor_copy(out=pr2s[:], in_=pr2[:])
                  prT_ps = prd.tile([128, 2], FP, tag="trps")
                  nc.tensor.transpose(prT_ps[:], pr2s[:], ident[0:2, 0:2])
                  prT = pewq.tile([128, 2], FP, tag="prT")
                  nc.vector.tensor_copy(out=prT[:], in_=prT_ps[:])

                  u = pewq.tile([128, 1], FP, tag="u")
                  nc.vector.tensor_mul(out=u[:], in0=prT[:, 1:2], in1=eT[:, b:b + 1])
                  v = pewq.tile([128, 1], FP, tag="v")
                  nc.vector.tensor_tensor(out=v[:], in0=prT[:, 0:1], in1=u[:],
                                          op=ALU.subtract)
                  t5 = pewq.tile([128, 1], FP, tag="t5")
                  nc.vector.tensor_mul(out=t5[:], in0=aT[:, b:b + 1], in1=ws128[:])
                  rcol = pewq.tile([128, 1], FP, tag="rcol")
                  nc.vector.tensor_add(out=rcol[:], in0=v[:], in1=t5[:])
                  nc.vector.tensor_copy(out=combT[:, KH, b:b + 1], in_=rcol[:])

        # ---------------- phase OUT -----------------------------------------
        if stop_phase >= 7:
          with contextlib.ExitStack() as ph:
              pw = ph.enter_context(tc.tile_pool(name="pw_o", bufs=1))
              pps_o = ph.enter_context(tc.tile_pool(name="pps_o", bufs=1,
                                                    space="PSUM"))
              woutT = pw.tile([128, KH + 1, I], BF)
              for k in range(KH + 1):
                  nc.scalar.dma_start_transpose(
                      woutT[:, k, :], woutbf_d[:, k * 128:(k + 1) * 128])
              po = pps_o.tile([BC, I], FP, tag="po")
              for k in range(KH + 1):
                  nc.tensor.matmul(po[:], combT[:, k, :], woutT[:, k, :],
                                   start=(k == 0),
                                   stop=(k == KH and "bout" not in nzb))
              if "bout" in nzb:
                  bias_mm(po[:], bias_t["bout"][:], BC)
              ob = pw.tile([BC, I], FP, tag="ob")
              nc.vector.tensor_copy(out=ob[:], in_=po[:])
              nc.sync.dma_start(out=out_d[:], in_=ob[:])
              if debug:
                  rTdbg = pw.tile([128, BC], FP, tag="rTdbg")
                  nc.vector.tensor_copy(out=rTdbg[:], in_=combT[:, KH, :])
                  nc.sync.dma_start(out=dbg_r_d[:], in_=rTdbg[:])
        else:
            zo = ptmp.tile([BC, I], FP, tag='zo')
            nc.vector.memset(zo[:], 0.0)
            nc.sync.dma_start(out=out_d[:], in_=zo[:])
    nc.compile()
    return nc


_NC_CACHE = {}


def _get_nc(S, nzb_key, debug):
    sp = int(os.environ.get('BASSGRU_STOP', '7'))
    key = (S, nzb_key, debug, sp, os.environ.get("BASSGRU_SUB", "9"),
           os.environ.get("BASSGRU_SUBSTEP", "9"))
    if key not in _NC_CACHE:
        _NC_CACHE[key] = build_nc(S=S, nonzero_biases=nzb_key, debug=debug, stop_phase=sp)
    return _NC_CACHE[key]


def make_in_maps(inputs, S=S_FULL, debug=False):
    f32 = lambda a: np.ascontiguousarray(np.asarray(a), dtype=np.float32)
    bias_names = {"bih0": "b_ih0", "bhh0": "b_hh0", "bih1": "b_ih1",
                  "bhh1": "b_hh1", "bk": "bk", "bbeta": "bbeta", "bg": "bg",
                  "bgamma": "bgamma", "be": "be", "ba": "ba", "bout": "bout"}
    nzb = tuple(sorted(k for k, src in bias_names.items()
                       if np.any(np.asarray(inputs[src]) != 0)))
    nc = _get_nc(S, nzb, debug)
    shared = {nm: f32(inputs[nm]) for nm in
              ["W_ih0", "W_hh0", "W_ih1", "W_hh1", "Wk", "Wbeta", "Wg",
               "Wgamma", "We", "Wa", "Wout"]}
    for k, src in bias_names.items():
        if k in nzb:
            shared[k] = f32(inputs[src]).reshape(1, -1)
    x = f32(inputs["x"])
    mem = f32(inputs["memory"])
    wp = f32(inputs["w_prev"])
    in_maps = []
    for c in range(NCORES):
        m = dict(shared)
        m["x"] = x[c * BC:(c + 1) * BC]
        m["memory"] = mem[c * BC:(c + 1) * BC]
        m["w_prev"] = wp[c * BC:(c + 1) * BC]
        in_maps.append(m)
    return nc, in_maps, nzb


def kernel(**inputs) -> np.ndarray:
    debug = bool(int(os.environ.get("BASSGRU_DEBUG", "0")))
    S = int(os.environ.get("BASSGRU_S", str(S_FULL)))
    nc, in_maps, _ = make_in_maps(inputs, S=S, debug=debug)
    res = run_bass_kernel_spmd(nc, in_maps, list(range(NCORES)))
    outs = [res.results[c]["out"] for c in range(NCORES)]
    if debug:
        kernel.last_results = res.results
    return np.concatenate(outs, axis=0).astype(np.float32)
